# revision 3
# baseline (speedup 1.0000x reference)
"""CenterLoss kernel for 8 Trainium2 NeuronCores (Bass/Tile).

Problem: nn_CenterLoss (B = NUM_CLASSES = 16384, D = 1024, alpha = 0.5).

    delta[j]   = alpha * (centers[y[j]] - y_pred[j]) / (counts[y[j]] + 1)
    new_c      = centers - delta                      (elementwise, B == C)
    loss       = mean((y_pred - new_c[y])^2)

Per-row algebra (j1 = y, j2 = y[y], cnt2 = counts[j2], s2 = alpha/(cnt2+1)):

    diff[i] = (y_pred[i] - centers[j1[i]]) - s2[i]*(y_pred[j1[i]] - centers[j2[i]])
    loss    = mean(diff^2)

Sharding: data-parallel over the batch dim; each of the 8 cores processes
2048 rows, gathering rows of the replicated full y_pred / centers via
indirect DMA. Host does integer index prep (bincount / index composition)
and the final 1024-element partial-sum reduction; all floating-point work
on the 16384x1024 tensors runs on device.
"""

import sys

import numpy as np

for _p in ("/opt/trn_rl_repo", "/root/.axon_site/_ro/trn_rl_repo"):
    if _p not in sys.path:
        sys.path.append(_p)

from concourse import bass, mybir
from concourse.tile import TileContext
from concourse.bass_utils import run_bass_kernel_spmd

B = 16384
D = 1024
P = 128
NCORES = 8
SH = B // NCORES  # rows per core
T = SH // P       # 128-row tiles per core
ALPHA = 0.5

F32 = mybir.dt.float32
I32 = mybir.dt.int32


def _split_sync_waits(nc, max_waits: int = 1):
    """walrus in this container rejects >~2 sync waits per instruction
    ("Too many sync wait commands"); hoist excess waits onto same-engine
    nops placed immediately before the instruction."""
    ctr = 0
    for f in nc.m.functions:
        for bb in f.blocks:
            new_insts = []
            for inst in bb.instructions:
                si = getattr(inst, "sync_info", None)
                waits = list(si.on_wait) if si is not None and si.on_wait else []
                if len(waits) > max_waits:
                    rest = waits[max_waits:]
                    si.on_wait = waits[:max_waits]
                    for k in range(0, len(rest), max_waits):
                        nop = mybir.InstNoOp(name=f"WSPLIT-{ctr}")
                        ctr += 1
                        nop.engine = inst.engine
                        nop.sync_info = mybir.SyncInfo(
                            on_wait=list(rest[k : k + max_waits]), on_update=[]
                        )
                        new_insts.append(nop)
                new_insts.append(inst)
            bb.instructions[:] = new_insts
    return nc


def _build_nc(split_waits=True):
    nc = bass.Bass()
    yp_shard = nc.dram_tensor("yp_shard", [SH, D], F32, kind="ExternalInput")
    yp_full = nc.dram_tensor("yp_full", [B, D], F32, kind="ExternalInput")
    cent_full = nc.dram_tensor("cent_full", [B, D], F32, kind="ExternalInput")
    # index/scale tables, laid out [P, T]: column t serves tile t
    j1 = nc.dram_tensor("j1", [P, T], I32, kind="ExternalInput")
    j2 = nc.dram_tensor("j2", [P, T], I32, kind="ExternalInput")
    cnt2 = nc.dram_tensor("cnt2", [P, T], F32, kind="ExternalInput")
    partial = nc.dram_tensor("partial", [P, 1], F32, kind="ExternalOutput")

    with TileContext(nc) as tc:
        with (
            tc.tile_pool(name="idx", bufs=1) as idxp,
            tc.tile_pool(name="big", bufs=3) as bigp,
            tc.tile_pool(name="work", bufs=3) as workp,
            tc.tile_pool(name="accp", bufs=1) as accp,
        ):
            j1_sb = idxp.tile([P, T], I32)
            nc.sync.dma_start(out=j1_sb[:], in_=j1[:])
            j2_sb = idxp.tile([P, T], I32)
            nc.sync.dma_start(out=j2_sb[:], in_=j2[:])
            cnt_sb = idxp.tile([P, T], F32)
            nc.sync.dma_start(out=cnt_sb[:], in_=cnt2[:])
            # s2 = ALPHA / (cnt2 + 1)
            s2_sb = idxp.tile([P, T], F32)
            nc.vector.tensor_scalar_add(s2_sb[:], cnt_sb[:], 1.0)
            nc.vector.reciprocal(s2_sb[:], s2_sb[:])
            nc.vector.tensor_scalar_mul(s2_sb[:], s2_sb[:], ALPHA)

            acc = accp.tile([P, 1], F32)
            nc.vector.memset(acc[:], 0.0)

            for t in range(T):
                yp_t = bigp.tile([P, D], F32, tag="yp")
                cj1_t = bigp.tile([P, D], F32, tag="cj1")
                ypj1_t = bigp.tile([P, D], F32, tag="ypj1")
                cj2_t = bigp.tile([P, D], F32, tag="cj2")
                nc.sync.dma_start(out=yp_t[:], in_=yp_shard[t * P : (t + 1) * P, :])
                nc.gpsimd.indirect_dma_start(
                    out=cj1_t[:],
                    out_offset=None,
                    in_=cent_full[:],
                    in_offset=bass.IndirectOffsetOnAxis(ap=j1_sb[:, t : t + 1], axis=0),
                )
                nc.gpsimd.indirect_dma_start(
                    out=ypj1_t[:],
                    out_offset=None,
                    in_=yp_full[:],
                    in_offset=bass.IndirectOffsetOnAxis(ap=j1_sb[:, t : t + 1], axis=0),
                )
                nc.gpsimd.indirect_dma_start(
                    out=cj2_t[:],
                    out_offset=None,
                    in_=cent_full[:],
                    in_offset=bass.IndirectOffsetOnAxis(ap=j2_sb[:, t : t + 1], axis=0),
                )

                # u = y_pred[j1] - centers[j2]
                u = workp.tile([P, D], F32, tag="u")
                nc.vector.tensor_tensor(
                    out=u[:], in0=ypj1_t[:], in1=cj2_t[:], op=mybir.AluOpType.subtract
                )
                # v = yp_self - centers[j1]
                v = workp.tile([P, D], F32, tag="v")
                nc.vector.tensor_tensor(
                    out=v[:], in0=yp_t[:], in1=cj1_t[:], op=mybir.AluOpType.subtract
                )
                # ndiff = s2*u - v  (= -diff; sign washes out in the square)
                nd = workp.tile([P, D], F32, tag="nd")
                nc.vector.scalar_tensor_tensor(
                    out=nd[:],
                    in0=u[:],
                    scalar=s2_sb[:, t : t + 1],
                    in1=v[:],
                    op0=mybir.AluOpType.mult,
                    op1=mybir.AluOpType.subtract,
                )
                # sq = ndiff^2, rowsum = sum(sq) per partition
                sq = workp.tile([P, D], F32, tag="sq")
                rowsum = workp.tile([P, 1], F32, tag="rowsum")
                nc.scalar.activation(
                    out=sq[:],
                    in_=nd[:],
                    func=mybir.ActivationFunctionType.Square,
                    accum_out=rowsum[:],
                )
                nc.vector.tensor_tensor(
                    out=acc[:], in0=acc[:], in1=rowsum[:], op=mybir.AluOpType.add
                )

            nc.sync.dma_start(out=partial[:], in_=acc[:])

    if split_waits:
        _split_sync_waits(nc)
    return nc


_NC_CACHE = {}


def _get_nc(split_waits=True):
    key = ("nc", split_waits)
    if key not in _NC_CACHE:
        _NC_CACHE[key] = _build_nc(split_waits=split_waits)
    return _NC_CACHE[key]


def make_in_maps(y_true, y_pred, centers):
    y_true = np.asarray(y_true, dtype=np.int64)
    yp = np.ascontiguousarray(np.asarray(y_pred), dtype=np.float32)
    cent = np.ascontiguousarray(np.asarray(centers), dtype=np.float32)

    counts = np.bincount(y_true, minlength=B)
    j1 = y_true.astype(np.int32)
    j2 = y_true[y_true].astype(np.int32)
    cnt2 = counts[j2].astype(np.float32)

    in_maps = []
    for c in range(NCORES):
        sl = slice(c * SH, (c + 1) * SH)
        in_maps.append(
            {
                "yp_shard": yp[sl],
                "yp_full": yp,
                "cent_full": cent,
                "j1": np.ascontiguousarray(j1[sl].reshape(T, P).T),
                "j2": np.ascontiguousarray(j2[sl].reshape(T, P).T),
                "cnt2": np.ascontiguousarray(cnt2[sl].reshape(T, P).T),
            }
        )
    return in_maps


def kernel(y_true, y_pred, centers):
    nc = _get_nc()
    in_maps = make_in_maps(y_true, y_pred, centers)
    res = run_bass_kernel_spmd(nc, in_maps, core_ids=list(range(NCORES)))
    total = np.float64(0.0)
    for c in range(NCORES):
        total += res.results[c]["partial"].astype(np.float64).sum()
    return np.float32(total / (B * D))
